# revision 17
# baseline (speedup 1.0000x reference)
"""Trainium2 Bass kernel for a GPT-style transformer block (B=2, T=2048, C=768,
NH=12, HD=64, DFF=3072), distributed over 8 NeuronCores.

Sharding: token-data-parallel with zigzag strip assignment, zero collectives.
  - cores 0-3 process batch 0, cores 4-7 batch 1.
  - within a batch, rank r owns token strips r and 7-r (strips of 256 tokens).
  - each core redundantly computes K/V for tokens [0, 256*(8-r)) (its causal
    prefix), so no cross-core communication is needed at all.
4 distinct per-rank programs are compiled and dispatched concurrently to the 8
devices via async PJRT.

All matmuls run in float32r (full-rate fp32 mode, ~1e-3 matmul accuracy).
LayerNorm affine params are folded into the adjacent weights on the host.
Attention uses exp-without-max softmax (safe for this distribution) computed in
a transposed layout so softmax normalization folds into the PSUM eviction.
"""

import sys
import types
import functools

sys.path.insert(0, "/opt/trn_rl_repo")

# ---- antenv.axon_hooks shim (missing module in this image) -----------------
if "antenv.axon_hooks" not in sys.modules:
    _hooks = types.ModuleType("antenv.axon_hooks")
    _hooks._hook = None
    _hooks.set_axon_ntff_profile_hook = lambda h: setattr(_hooks, "_hook", h)
    _hooks.get_axon_ntff_profile_hook = lambda: _hooks._hook
    sys.modules["antenv.axon_hooks"] = _hooks
    try:
        import antenv

        antenv.axon_hooks = _hooks
    except ImportError:
        pass

import numpy as np
import jax

import concourse.bass as bass
import concourse.mybir as mybir
import concourse.tile as tile
from concourse import bacc
from concourse.bass2jax import (
    _bass_exec_p,
    install_neuronx_cc_hook,
    partition_id_tensor,
)
from concourse.masks import make_identity

B, T, C = 2, 2048, 768
NH, HD, DFF = 12, 64, 64 * 48  # DFF = 3072
STRIP = 256
F32 = mybir.dt.float32
F32R = mybir.dt.float32r
EPS = 1e-5


# ---------------------------------------------------------------------------
# Per-rank program builder
# ---------------------------------------------------------------------------
def build_rank_program(r: int):
    """Program for rank r (strips r and 7-r of one batch element)."""
    sA, sB = r, 7 - r
    NTK = 2 * (8 - r)          # kt tiles of 128 in the causal prefix
    T_kv = NTK * 128
    NB = 8 - r                 # 256-token blocks in the prefix
    # token-block indices that contain the own strips (block size == strip)
    blkA, blkB = sA, sB

    nc = bacc.Bacc("TRN2", target_bir_lowering=False, debug=False, num_devices=1)

    x_in = nc.declare_dram_parameter("x", [T, C], F32, isOutput=False)
    wq_in = nc.declare_dram_parameter("wq", [C, C], F32, isOutput=False)
    wk_in = nc.declare_dram_parameter("wk", [C, C], F32, isOutput=False)
    wv_in = nc.declare_dram_parameter("wv", [C, C], F32, isOutput=False)
    bq_in = nc.declare_dram_parameter("bq", [C], F32, isOutput=False)
    bk_in = nc.declare_dram_parameter("bk", [C], F32, isOutput=False)
    bv_in = nc.declare_dram_parameter("bv", [C], F32, isOutput=False)
    wcp_in = nc.declare_dram_parameter("wcp", [C, C], F32, isOutput=False)
    bcp_in = nc.declare_dram_parameter("bcp", [C], F32, isOutput=False)
    wfc_in = nc.declare_dram_parameter("wfc", [C, DFF], F32, isOutput=False)
    bfc_in = nc.declare_dram_parameter("bfc", [DFF], F32, isOutput=False)
    wpj_in = nc.declare_dram_parameter("wpj", [DFF, C], F32, isOutput=False)
    bpj_in = nc.declare_dram_parameter("bpj", [C], F32, isOutput=False)
    out_dram = nc.declare_dram_parameter("out", [512, C], F32, isOutput=True)

    with tile.TileContext(nc) as tc:
        _build_body(nc, tc, r, sA, sB, NTK, T_kv, NB, blkA, blkB,
                    x_in, wq_in, wk_in, wv_in, bq_in, bk_in, bv_in,
                    wcp_in, bcp_in, wfc_in, bfc_in, wpj_in, bpj_in, out_dram)
    nc.compile()
    return nc


def _build_body(nc, tc, r, sA, sB, NTK, T_kv, NB, blkA, blkB,
                x_in, wq_in, wk_in, wv_in, bq_in, bk_in, bv_in,
                wcp_in, bcp_in, wfc_in, bfc_in, wpj_in, bpj_in, out_dram):
    from contextlib import ExitStack

    ev_state = {"i": 0}

    def evict(out, in_):
        """PSUM->SBUF copy, alternating DVE/ACT to balance engine load."""
        i = ev_state["i"]
        ev_state["i"] += 1
        if i % 2 == 0:
            nc.vector.tensor_copy(out, in_)
        else:
            nc.scalar.copy(out, in_)

    cast_state = {"i": 0}

    def cast(out, in_):
        """fp32 -> fp32r rounding copy, alternating DVE/ACT."""
        i = cast_state["i"]
        cast_state["i"] += 1
        if i % 2 == 0:
            nc.scalar.copy(out, in_)
        else:
            nc.vector.tensor_copy(out, in_)

    R_ = F32R

    with ExitStack() as ctx:
        # ------- constants -------
        const = ctx.enter_context(tc.tile_pool(name="const", bufs=1))
        id_f = const.tile([128, 128], F32)
        make_identity(nc, id_f[:])
        id_r = const.tile([128, 128], F32R)
        nc.vector.tensor_copy(id_r[:], id_f[:])
        eps_t = const.tile([128, 1], F32)
        nc.vector.memset(eps_t[:], EPS)
        ones_col = const.tile([128, 6], F32)
        nc.vector.memset(ones_col[:], 1.0)
        # causal masks for the two in-strip kt chunk offsets: [128, 2, 256]
        mask_t = const.tile([128, 2, 256], F32)
        nc.vector.memset(mask_t[:], 1.0)
        for off in range(2):
            nc.gpsimd.affine_select(
                out=mask_t[:, off, :],
                in_=mask_t[:, off, :],
                compare_op=mybir.AluOpType.is_ge,
                fill=0.0,
                base=-128 * off,
                pattern=[[1, 256]],
                channel_multiplier=-1,
            )

        # ------- bias tiles -------
        # per-partition bias tiles [128, 6] (column j = head-pair j)
        bq_sb = const.tile([128, 6], F32)
        bk_sb = const.tile([128, 6], F32)
        for src, dst in ((bq_in, bq_sb), (bk_in, bk_sb)):
            nc.sync.dma_start(out=dst[:], in_=src[:].rearrange("(j p) -> p j", p=128))
        bfc_sb = const.tile([128, 24], F32)
        nc.sync.dma_start(out=bfc_sb[:], in_=bfc_in[:].rearrange("(f p) -> p f", p=128))
        # bias rows [1, C] (added via K=1 ones matmuls); rounded in place
        brow_f = const.tile([1, 3, C], F32)
        nc.sync.dma_start(out=brow_f[:, 0, :], in_=bv_in[:][None, :])
        nc.sync.dma_start(out=brow_f[:, 1, :], in_=bcp_in[:][None, :])
        nc.sync.dma_start(out=brow_f[:, 2, :], in_=bpj_in[:][None, :])
        # broadcast bias rows across partitions for free-dim bias adds
        bias_bc = const.tile([128, 3, C], F32)
        nc.gpsimd.partition_broadcast(bias_bc[:], brow_f[:])
        bv_bc = bias_bc[:, 0, :]
        bcp_bc = bias_bc[:, 1, :]
        bpj_bc = bias_bc[:, 2, :]

        # ------- activation tensors spanning stages 3-4 -------
        acts = ctx.enter_context(tc.tile_pool(name="acts", bufs=1))
        yT_sb = acts.tile([128, 6, 512], F32R)         # y cols x own q

        s123 = ctx.enter_context(ExitStack())
        acts13 = s123.enter_context(tc.tile_pool(name="acts13", bufs=1))
        hT_sb = acts13.tile([128, 6, T_kv], F32R)      # ln1(x) transposed
        qT_sb = acts13.tile([128, 6, 512], F32R)       # head-pair rows x own q

        # =================== stage 1: LN1 + transpose =======================
        with ExitStack() as s1:
            ln_pool = s1.enter_context(tc.tile_pool(name="ln", bufs=2))
            tp_ps = s1.enter_context(tc.tile_pool(name="tp_ps", bufs=3, space="PSUM"))
            for b2 in range(NB):
                x2_t = ln_pool.tile([128, 2, C], F32, tag="x")
                nc.sync.dma_start(
                    out=x2_t[:],
                    in_=x_in[b2 * 256:(b2 + 1) * 256, :].rearrange("(t p) c -> p t c", p=128))
                for tt in range(2):
                    ti = b2 * 2 + tt
                    x_t = x2_t[:, tt, :]
                    xg = x_t.rearrange("p (g d) -> p g d", g=3)
                    stats = ln_pool.tile([128, 3, 6], F32, tag="st")
                    for g in range(3):
                        nc.vector.bn_stats(out=stats[:, g, :], in_=xg[:, g, :])
                    mv = ln_pool.tile([128, 2], F32, tag="mv")
                    nc.vector.bn_aggr(out=mv[:], in_=stats[:])
                    rstd = ln_pool.tile([128, 1], F32, tag="rstd")
                    nc.scalar.activation(
                        out=rstd[:], in_=mv[:, 1:2],
                        func=mybir.ActivationFunctionType.Sqrt,
                        bias=eps_t[:], scale=1.0,
                    )
                    nc.vector.reciprocal(out=rstd[:], in_=rstd[:])
                    h_t = ln_pool.tile([128, C], F32R, tag="h")
                    nc.vector.tensor_scalar(
                        out=h_t[:], in0=x_t,
                        scalar1=mv[:, 0:1], scalar2=rstd[:],
                        op0=mybir.AluOpType.subtract, op1=mybir.AluOpType.mult,
                    )
                    for c in range(6):
                        pt = tp_ps.tile([128, 128], F32R, tag="tp")
                        nc.tensor.transpose(pt[:], h_t[:, c * 128:(c + 1) * 128], id_r[:])
                        evict(hT_sb[:, c, ti * 128:(ti + 1) * 128], pt[:])

        # ========== stages 2+3 per head-half: K/V/Q GEMMs + attention =======
        for ph in range(2):                 # heads 6*ph .. 6*ph+5
            with ExitStack() as s23:
                wkv_pool = s23.enter_context(tc.tile_pool(name="wkv", bufs=1))
                wq_pool = s23.enter_context(tc.tile_pool(name="wqs", bufs=2))
                kv_pool = s23.enter_context(tc.tile_pool(name="kv", bufs=1))
                att_pool = s23.enter_context(tc.tile_pool(name="att", bufs=3))
                nrm_pool = s23.enter_context(tc.tile_pool(name="nrm", bufs=2))
                mm_ps = s23.enter_context(tc.tile_pool(name="mm_ps", bufs=2, space="PSUM"))
                att_ps = s23.enter_context(tc.tile_pool(name="att_ps", bufs=2, space="PSUM"))
                yt_ps = s23.enter_context(tc.tile_pool(name="yt_ps", bufs=2, space="PSUM"))

                co = ph * 384               # column offset of this head-half
                wk_t = wkv_pool.tile([128, 6, 384], F32R, tag="wk")
                wv_t = wkv_pool.tile([128, 6, 384], F32R, tag="wv")
                with tc.tile_pool(name="wkvs", bufs=2) as wkvs_pool:
                    for src_in, dst in ((wk_in, wk_t), (wv_in, wv_t)):
                        wstg = wkvs_pool.tile([128, 6, 384], F32, tag="wkvs")
                        nc.sync.dma_start(
                            out=wstg[:],
                            in_=src_in[:, co:co + 384].rearrange("(c k) n -> k c n", k=128))
                        for c in range(6):
                            cast(dst[:, c, :], wstg[:, c, :])

                kT_sb = kv_pool.tile([128, 3, T_kv], F32R, tag="kT")
                v_sb = kv_pool.tile([128, NTK, 6, 65], F32R, tag="v")
                for ti in range(NTK):
                    nc.vector.tensor_copy(v_sb[:, ti, :, 64], ones_col[:])

                # K GEMM (N=512)
                nblk512 = [(i * 512, 512) for i in range(T_kv // 512)]
                if T_kv % 512:
                    nblk512.append((T_kv - T_kv % 512, T_kv % 512))
                for tb, bw in nblk512:
                    for j in range(3):
                        pk = mm_ps.tile([128, 512], F32, tag="pk")
                        for c in range(6):
                            nc.tensor.matmul(
                                pk[:, 0:bw], wk_t[:, c, j * 128:(j + 1) * 128],
                                hT_sb[:, c, tb:tb + bw],
                                start=(c == 0), stop=(c == 5),
                            )
                        nc.vector.tensor_scalar(
                            out=kT_sb[:, j, tb:tb + bw], in0=pk[:, 0:bw],
                            scalar1=bk_sb[:, 3 * ph + j:3 * ph + j + 1], scalar2=None,
                            op0=mybir.AluOpType.add,
                        )
                # V GEMM (natural layout)
                for ti in range(NTK):
                    pv = mm_ps.tile([128, 384], F32, tag="pv")
                    for c in range(6):
                        nc.tensor.matmul(
                            pv[:], hT_sb[:, c, ti * 128:(ti + 1) * 128],
                            wv_t[:, c, :],
                            start=(c == 0), stop=(c == 5),
                        )
                    nc.vector.tensor_tensor(
                        out=v_sb[:, ti, :, 0:64],
                        in0=pv[:].rearrange("p (h d) -> p h d", d=64),
                        in1=bv_bc[:, co:co + 384].rearrange("p (h d) -> p h d", d=64),
                        op=mybir.AluOpType.add,
                    )
                # Q GEMM for own strips (both strips in one N=512 matmul)
                tbA, tbB = blkA * 256, blkB * 256
                for j in range(3):
                    jj = 3 * ph + j
                    wq_s = wq_pool.tile([128, 6, 128], F32, tag="wqs")
                    nc.sync.dma_start(
                        out=wq_s[:],
                        in_=wq_in[:, jj * 128:(jj + 1) * 128].rearrange(
                            "(c k) n -> k c n", k=128))
                    wq_t = wq_pool.tile([128, 6, 128], F32R, tag="wqr")
                    for c in range(6):
                        cast(wq_t[:, c, :], wq_s[:, c, :])
                    pq = mm_ps.tile([128, 512], F32, tag="pk")
                    for c in range(6):
                        rhs = bass.AP(
                            tensor=hT_sb[:, c, :].tensor,
                            offset=hT_sb[:, c, tbA:tbA + 1].offset,
                            ap=[list(p) for p in hT_sb[:, c, :].ap[:1]]
                            + [[hT_sb[:, c, :].ap[-1][0] * (tbB - tbA), 2],
                               [hT_sb[:, c, :].ap[-1][0], 256]],
                        )
                        nc.tensor.matmul(
                            pq[:], wq_t[:, c, :], rhs,
                            start=(c == 0), stop=(c == 5),
                        )
                    nc.vector.tensor_scalar(
                        out=qT_sb[:, jj, :], in0=pq[:],
                        scalar1=bq_sb[:, jj:jj + 1], scalar2=None,
                        op0=mybir.AluOpType.add,
                    )

                # ---- attention for heads of this half ----
                # chunks < n_sh apply to both strips (N=512); rest strip-B only
                n_sh = 2 * (sA + 1)
                n_all = 2 * (sB + 1)
                for hh in range(6):
                    h = 6 * ph + hh
                    j, po = hh // 2, 64 * (hh % 2)
                    jj = h // 2
                    kT_h = kT_sb[po:po + 64, j, :]
                    qT_h = qT_sb[64 * (h % 2):64 * (h % 2) + 64, jj, :]
                    yt = yt_ps.tile([65, 512], F32, tag="yt")
                    for kc in range(n_all):
                        shared = kc < n_sh
                        ww = 512 if shared else 256
                        qs = 0 if shared else 256
                        pa = att_ps.tile([128, 512], F32, tag="pa")
                        nc.tensor.matmul(
                            pa[:, 0:ww], kT_h[:, kc * 128:(kc + 1) * 128],
                            qT_h[:, qs:qs + ww],
                            start=True, stop=True,
                        )
                        at = att_pool.tile([128, 512], F32R, tag="at")
                        nc.scalar.activation(
                            out=at[:, 0:ww], in_=pa[:, 0:ww],
                            func=mybir.ActivationFunctionType.Exp,
                        )
                        if kc in (2 * sA, 2 * sA + 1):
                            nc.vector.tensor_mul(
                                at[:, 0:256], at[:, 0:256], mask_t[:, kc - 2 * sA, :])
                        if kc in (2 * sB, 2 * sB + 1):
                            # strip-B columns live at 256:512 when shared, 0:256 when B-only
                            boff = 256 if shared else 0
                            nc.vector.tensor_mul(
                                at[:, boff:boff + 256],
                                at[:, boff:boff + 256],
                                mask_t[:, kc - 2 * sB, :])
                        nc.tensor.matmul(
                            yt[0:65, qs:qs + ww], v_sb[:, kc, hh, 0:65], at[:, 0:ww],
                            start=(kc == 0), stop=(kc == n_all - 1),
                            skip_group_check=True,
                        )
                    sume = nrm_pool.tile([1, 512], F32, tag="sume")
                    nc.vector.tensor_copy(sume[:], yt[64:65, :])
                    bcast = nrm_pool.tile([64, 512], F32, tag="bcast")
                    nc.gpsimd.partition_broadcast(bcast[:], sume[:])
                    nc.vector.reciprocal(out=bcast[:], in_=bcast[:])
                    nc.vector.tensor_mul(
                        yT_sb[po:po + 64, jj, :], yt[0:64, :], bcast[:],
                    )

        s123.close()  # free hT/qT SBUF before the MLP stages

        # =================== stages 4-6: c_proj, MLP ========================
        with ExitStack() as s46:
            wcp_pool = s46.enter_context(tc.tile_pool(name="wcp", bufs=1))
            act46 = s46.enter_context(tc.tile_pool(name="act46", bufs=1))
            ln2_pool = s46.enter_context(tc.tile_pool(name="ln2", bufs=2))
            stream_pool = s46.enter_context(tc.tile_pool(name="stream", bufs=2))
            out_pool = s46.enter_context(tc.tile_pool(name="outp", bufs=3))

            # c_proj weights resident fp32r
            wcp_t = wcp_pool.tile([128, 6, C], F32R)
            with tc.tile_pool(name="wcps", bufs=1) as wcps_pool:
                wstg = wcps_pool.tile([128, 6, C], F32, tag="wcps")
                nc.sync.dma_start(out=wstg[:], in_=wcp_in[:].rearrange("(j k) n -> k j n", k=128))
                for j in range(6):
                    cast(wcp_t[:, j, :], wstg[:, j, :])

            x1_sb = act46.tile([128, 4, C], F32)
            h2T_sb = act46.tile([128, 6, 512], F32R)
            gT_sb = act46.tile([128, 24, 512], F32R)

            own_rows = (sA * 256, sA * 256 + 128, sB * 256, sB * 256 + 128)
            # ---- stage 4: c_proj + residual + LN2 + transpose ----
            s4 = ExitStack()
            tp2_ps = s4.enter_context(tc.tile_pool(name="tp2_ps", bufs=2, space="PSUM"))
            cp_ps = s4.enter_context(tc.tile_pool(name="cp_ps", bufs=2, space="PSUM"))
            for m in range(4):
                pp = []
                for i in range(2):
                    pp_i = cp_ps.tile([128, 384], F32, tag=f"cp{i}")
                    pp.append(pp_i)
                for half in range(2):
                    for j in range(6):
                        nc.tensor.matmul(
                            pp[half][:],
                            yT_sb[:, j, m * 128:(m + 1) * 128],
                            wcp_t[:, j, half * 384:(half + 1) * 384],
                            start=(j == 0), stop=(j == 5),
                        )
                x_own = ln2_pool.tile([128, C], F32, tag="xo")
                nc.sync.dma_start(out=x_own[:], in_=x_in[own_rows[m]:own_rows[m] + 128, :])
                nc.vector.tensor_add(x_own[:], x_own[:], bcp_bc[:])
                for half in range(2):
                    nc.vector.tensor_add(
                        x1_sb[:, m, half * 384:(half + 1) * 384],
                        pp[half][:], x_own[:, half * 384:(half + 1) * 384],
                    )
                # LN2
                x1g = x1_sb[:, m, :].rearrange("p (g d) -> p g d", g=3)
                stats = ln2_pool.tile([128, 3, 6], F32, tag="st2")
                for g in range(3):
                    nc.vector.bn_stats(out=stats[:, g, :], in_=x1g[:, g, :])
                mv = ln2_pool.tile([128, 2], F32, tag="mv2")
                nc.vector.bn_aggr(out=mv[:], in_=stats[:])
                rstd = ln2_pool.tile([128, 1], F32, tag="rstd2")
                nc.scalar.activation(
                    out=rstd[:], in_=mv[:, 1:2],
                    func=mybir.ActivationFunctionType.Sqrt,
                    bias=eps_t[:], scale=1.0,
                )
                nc.vector.reciprocal(out=rstd[:], in_=rstd[:])
                h2 = ln2_pool.tile([128, C], F32R, tag="h2")
                nc.vector.tensor_scalar(
                    out=h2[:], in0=x1_sb[:, m, :],
                    scalar1=mv[:, 0:1], scalar2=rstd[:],
                    op0=mybir.AluOpType.subtract, op1=mybir.AluOpType.mult,
                )
                for c in range(6):
                    pt = tp2_ps.tile([128, 128], F32R, tag="tp2")
                    nc.tensor.transpose(pt[:], h2[:, c * 128:(c + 1) * 128], id_r[:])
                    evict(h2T_sb[:, c, m * 128:(m + 1) * 128], pt[:])

            s4.close()
            # ---- stage 5: fc + gelu ----
            s5 = ExitStack()
            pf_ps = s5.enter_context(tc.tile_pool(name="pf_ps", bufs=3, space="PSUM"))
            for f in range(24):
                wfc_s = stream_pool.tile([128, 6, 128], F32, tag="wfc_s")
                nc.sync.dma_start(
                    out=wfc_s[:],
                    in_=wfc_in[:, f * 128:(f + 1) * 128].rearrange("(c k) n -> k c n", k=128),
                )
                wfc_t = stream_pool.tile([128, 6, 128], F32R, tag="wfc_r")
                for c in range(6):
                    cast(wfc_t[:, c, :], wfc_s[:, c, :])
                pf = pf_ps.tile([128, 512], F32, tag="pf")
                for c in range(6):
                    nc.tensor.matmul(
                        pf[:], wfc_t[:, c, :], h2T_sb[:, c, :],
                        start=(c == 0), stop=(c == 5),
                    )
                nc.scalar.activation(
                    out=gT_sb[:, f, :], in_=pf[:],
                    func=mybir.ActivationFunctionType.Gelu_apprx_tanh,
                    bias=bfc_sb[:, f:f + 1], scale=1.0,
                )

            s5.close()
            # ---- stage 6: proj + residual + store ----
            s6 = ExitStack()
            pj_ps = s6.enter_context(tc.tile_pool(name="pj_ps", bufs=1, space="PSUM"))
            for mp in range(2):
                pj = []
                for i in range(4):
                    pj_i = pj_ps.tile([128, 384], F32, tag=f"pj{i}")
                    pj.append(pj_i)
                for f in range(24):
                    wpj_s = stream_pool.tile([128, C], F32, tag="wpj_s")
                    nc.sync.dma_start(out=wpj_s[:], in_=wpj_in[f * 128:(f + 1) * 128, :])
                    wpj_t = stream_pool.tile([128, C], F32R, tag="wpj_r")
                    cast(wpj_t[:, 0:384], wpj_s[:, 0:384])
                    cast(wpj_t[:, 384:768], wpj_s[:, 384:768])
                    for m2 in range(2):
                        m = mp * 2 + m2
                        for half in range(2):
                            nc.tensor.matmul(
                                pj[m2 * 2 + half][:],
                                gT_sb[:, f, m * 128:(m + 1) * 128],
                                wpj_t[:, half * 384:(half + 1) * 384],
                                start=(f == 0), stop=(f == 23),
                            )
                for m2 in range(2):
                    m = mp * 2 + m2
                    o_t = out_pool.tile([128, C], F32, tag="o")
                    for half in range(2):
                        nc.vector.tensor_add(
                            o_t[:, half * 384:(half + 1) * 384],
                            pj[m2 * 2 + half][:],
                            x1_sb[:, m, half * 384:(half + 1) * 384],
                        )
                    nc.vector.tensor_add(o_t[:], o_t[:], bpj_bc[:])
                    nc.sync.dma_start(out=out_dram[m * 128:(m + 1) * 128, :], in_=o_t[:])
            s6.close()


# ---------------------------------------------------------------------------
# Runner
# ---------------------------------------------------------------------------
def _make_runner(nc):
    partition_name = nc.partition_id_tensor.name if nc.partition_id_tensor else None
    in_names, out_names, out_avals, zero_outs = [], [], [], []
    for alloc in nc.m.functions[0].allocations:
        if not isinstance(alloc, mybir.MemoryLocationSet):
            continue
        name = alloc.memorylocations[0].name
        if alloc.kind == "ExternalInput":
            if name != partition_name:
                in_names.append(name)
        elif alloc.kind == "ExternalOutput":
            out_names.append(name)
            shape = tuple(alloc.tensor_shape)
            dtype = mybir.dt.np(alloc.dtype)
            out_avals.append(jax.core.ShapedArray(shape, dtype))
            zero_outs.append(np.zeros(shape, dtype))
    n_params = len(in_names)
    all_names = list(in_names) + list(out_names)
    if partition_name is not None:
        all_names.append(partition_name)

    def _body(*args):
        operands = list(args)
        if partition_name is not None:
            operands.append(partition_id_tensor())
        outs = _bass_exec_p.bind(
            *operands,
            out_avals=tuple(out_avals),
            in_names=tuple(all_names),
            out_names=tuple(out_names),
            lowering_input_output_aliases=(),
            sim_require_finite=True,
            sim_require_nnan=True,
            nc=nc,
        )
        return tuple(outs)

    donate = tuple(range(n_params, n_params + len(out_names)))
    jitted = jax.jit(_body, donate_argnums=donate, keep_unused=True)
    return jitted, in_names, out_names, zero_outs


@functools.lru_cache(maxsize=None)
def _get_runners():
    install_neuronx_cc_hook()
    runners = []
    for r in range(4):
        nc = build_rank_program(r)
        runners.append(_make_runner(nc))
    return runners


def _prep_core_inputs(x, ln1_w, ln1_b, c_attn_w, c_attn_b, c_proj_w, c_proj_b,
                      ln2_w, ln2_b, fc_w, fc_b, proj_w, proj_b):
    """Fold LN affines into weights; split qkv. Returns shared weight dict."""
    f32 = np.float32
    wqkv = (ln1_w[:, None] * c_attn_w).astype(f32)
    bqkv = (c_attn_b + ln1_b @ c_attn_w).astype(f32)
    scale = f32(1.0 / np.sqrt(HD))
    shared = {
        "wq": np.ascontiguousarray(wqkv[:, 0:C] * scale),
        "wk": np.ascontiguousarray(wqkv[:, C:2 * C]),
        "wv": np.ascontiguousarray(wqkv[:, 2 * C:3 * C]),
        "bq": np.ascontiguousarray(bqkv[0:C] * scale),
        "bk": np.ascontiguousarray(bqkv[C:2 * C]),
        "bv": np.ascontiguousarray(bqkv[2 * C:3 * C]),
        "wcp": np.ascontiguousarray(c_proj_w.astype(f32)),
        "bcp": np.ascontiguousarray(c_proj_b.astype(f32)),
        "wfc": np.ascontiguousarray((ln2_w[:, None] * fc_w).astype(f32)),
        "bfc": np.ascontiguousarray((fc_b + ln2_b @ fc_w).astype(f32)),
        "wpj": np.ascontiguousarray(proj_w.astype(f32)),
        "bpj": np.ascontiguousarray(proj_b.astype(f32)),
    }
    return shared


def _dispatch_all(inputs):
    """Dispatch the 8 per-core executions asynchronously; return futures."""
    runners = _get_runners()
    devices = jax.devices()
    shared = _prep_core_inputs(**{k: np.asarray(v) for k, v in inputs.items()})
    x = np.asarray(inputs["x"], dtype=np.float32)
    futs = []
    for c in range(8):
        b, r = c // 4, c % 4
        jitted, in_names, out_names, zero_outs = runners[r]
        dev = devices[c]
        per_core = dict(shared)
        per_core["x"] = np.ascontiguousarray(x[b])
        args = [jax.device_put(per_core[n], dev) for n in in_names]
        args += [jax.device_put(z, dev) for z in zero_outs]
        futs.append((c, out_names, jitted(*args)))
    return futs


def kernel(**inputs) -> np.ndarray:
    futs = _dispatch_all(inputs)
    out = np.empty((B, T, C), dtype=np.float32)
    for c, out_names, fut in futs:
        b, r = c // 4, c % 4
        res = np.asarray(fut[out_names.index("out")])
        out[b, 256 * r:256 * r + 256] = res[0:256]
        out[b, 256 * (7 - r):256 * (7 - r) + 256] = res[256:512]
    return out


# revision 18
# speedup vs baseline: 1.1357x; 1.1357x over previous
"""Trainium2 Bass kernel for a GPT-style transformer block (B=2, T=2048, C=768,
NH=12, HD=64, DFF=3072), distributed over 8 NeuronCores.

Sharding: token-data-parallel with zigzag strip assignment, zero collectives.
  - cores 0-3 process batch 0, cores 4-7 batch 1.
  - within a batch, rank r owns token strips r and 7-r (strips of 256 tokens).
  - each core redundantly computes K/V for tokens [0, 256*(8-r)) (its causal
    prefix), so no cross-core communication is needed at all.
4 distinct per-rank programs are compiled and dispatched concurrently to the 8
devices via async PJRT.

All matmuls run in float32r (full-rate fp32 mode, ~1e-3 matmul accuracy).
LayerNorm affine params are folded into the adjacent weights on the host.
Attention uses exp-without-max softmax (safe for this distribution) computed in
a transposed layout so softmax normalization folds into the PSUM eviction.
"""

import sys
import types
import functools

sys.path.insert(0, "/opt/trn_rl_repo")

# ---- antenv.axon_hooks shim (missing module in this image) -----------------
if "antenv.axon_hooks" not in sys.modules:
    _hooks = types.ModuleType("antenv.axon_hooks")
    _hooks._hook = None
    _hooks.set_axon_ntff_profile_hook = lambda h: setattr(_hooks, "_hook", h)
    _hooks.get_axon_ntff_profile_hook = lambda: _hooks._hook
    sys.modules["antenv.axon_hooks"] = _hooks
    try:
        import antenv

        antenv.axon_hooks = _hooks
    except ImportError:
        pass

import numpy as np
import jax

import concourse.bass as bass
import concourse.mybir as mybir
import concourse.tile as tile
from concourse import bacc
from concourse.bass2jax import (
    _bass_exec_p,
    install_neuronx_cc_hook,
    partition_id_tensor,
)
from concourse.masks import make_identity

B, T, C = 2, 2048, 768
NH, HD, DFF = 12, 64, 64 * 48  # DFF = 3072
STRIP = 256
F32 = mybir.dt.float32
F32R = mybir.dt.float32r
EPS = 1e-5


# ---------------------------------------------------------------------------
# Per-rank program builder
# ---------------------------------------------------------------------------
def build_rank_program(r: int):
    """Program for rank r (strips r and 7-r of one batch element)."""
    sA, sB = r, 7 - r
    NTK = 2 * (8 - r)          # kt tiles of 128 in the causal prefix
    T_kv = NTK * 128
    NB = 8 - r                 # 256-token blocks in the prefix
    # token-block indices that contain the own strips (block size == strip)
    blkA, blkB = sA, sB

    nc = bacc.Bacc("TRN2", target_bir_lowering=False, debug=False, num_devices=1)

    x_in = nc.declare_dram_parameter("x", [T, C], F32, isOutput=False)
    wq_in = nc.declare_dram_parameter("wq", [C, C], F32, isOutput=False)
    wk_in = nc.declare_dram_parameter("wk", [C, C], F32, isOutput=False)
    wv_in = nc.declare_dram_parameter("wv", [C, C], F32, isOutput=False)
    bq_in = nc.declare_dram_parameter("bq", [C], F32, isOutput=False)
    bk_in = nc.declare_dram_parameter("bk", [C], F32, isOutput=False)
    bv_in = nc.declare_dram_parameter("bv", [C], F32, isOutput=False)
    wcp_in = nc.declare_dram_parameter("wcp", [C, C], F32, isOutput=False)
    bcp_in = nc.declare_dram_parameter("bcp", [C], F32, isOutput=False)
    wfc_in = nc.declare_dram_parameter("wfc", [C, DFF], F32, isOutput=False)
    bfc_in = nc.declare_dram_parameter("bfc", [DFF], F32, isOutput=False)
    wpj_in = nc.declare_dram_parameter("wpj", [DFF, C], F32, isOutput=False)
    bpj_in = nc.declare_dram_parameter("bpj", [C], F32, isOutput=False)
    out_dram = nc.declare_dram_parameter("out", [512, C], F32, isOutput=True)

    with tile.TileContext(nc) as tc:
        _build_body(nc, tc, r, sA, sB, NTK, T_kv, NB, blkA, blkB,
                    x_in, wq_in, wk_in, wv_in, bq_in, bk_in, bv_in,
                    wcp_in, bcp_in, wfc_in, bfc_in, wpj_in, bpj_in, out_dram)
    nc.compile()
    return nc


def _build_body(nc, tc, r, sA, sB, NTK, T_kv, NB, blkA, blkB,
                x_in, wq_in, wk_in, wv_in, bq_in, bk_in, bv_in,
                wcp_in, bcp_in, wfc_in, bfc_in, wpj_in, bpj_in, out_dram):
    from contextlib import ExitStack

    ev_state = {"i": 0}

    def evict(out, in_):
        """PSUM->SBUF copy, alternating DVE/ACT to balance engine load."""
        i = ev_state["i"]
        ev_state["i"] += 1
        if i % 2 == 0:
            nc.vector.tensor_copy(out, in_)
        else:
            nc.scalar.copy(out, in_)

    cast_state = {"i": 0}

    def cast(out, in_):
        """fp32 -> fp32r rounding copy, alternating DVE/ACT."""
        i = cast_state["i"]
        cast_state["i"] += 1
        if i % 2 == 0:
            nc.scalar.copy(out, in_)
        else:
            nc.vector.tensor_copy(out, in_)

    R_ = F32R

    with ExitStack() as ctx:
        # ------- constants -------
        const = ctx.enter_context(tc.tile_pool(name="const", bufs=1))
        id_f = const.tile([128, 128], F32)
        make_identity(nc, id_f[:])
        id_r = const.tile([128, 128], F32R)
        nc.vector.tensor_copy(id_r[:], id_f[:])
        eps_t = const.tile([128, 1], F32)
        nc.vector.memset(eps_t[:], EPS)
        ones_col = const.tile([128, 6], F32)
        nc.vector.memset(ones_col[:], 1.0)
        # causal masks for the two in-strip kt chunk offsets: [128, 2, 256]
        mask_t = const.tile([128, 2, 256], F32)
        nc.vector.memset(mask_t[:], 1.0)
        for off in range(2):
            nc.gpsimd.affine_select(
                out=mask_t[:, off, :],
                in_=mask_t[:, off, :],
                compare_op=mybir.AluOpType.is_ge,
                fill=0.0,
                base=-128 * off,
                pattern=[[1, 256]],
                channel_multiplier=-1,
            )

        # ------- bias tiles -------
        # per-partition bias tiles [128, 6] (column j = head-pair j)
        bq_sb = const.tile([128, 6], F32)
        bk_sb = const.tile([128, 6], F32)
        for src, dst in ((bq_in, bq_sb), (bk_in, bk_sb)):
            nc.sync.dma_start(out=dst[:], in_=src[:].rearrange("(j p) -> p j", p=128))
        bfc_sb = const.tile([128, 24], F32)
        nc.sync.dma_start(out=bfc_sb[:], in_=bfc_in[:].rearrange("(f p) -> p f", p=128))
        # bias rows [1, C] (added via K=1 ones matmuls); rounded in place
        brow_f = const.tile([1, 3, C], F32)
        nc.sync.dma_start(out=brow_f[:, 0, :], in_=bv_in[:][None, :])
        nc.sync.dma_start(out=brow_f[:, 1, :], in_=bcp_in[:][None, :])
        nc.sync.dma_start(out=brow_f[:, 2, :], in_=bpj_in[:][None, :])
        # broadcast bias rows across partitions for free-dim bias adds
        bias_bc = const.tile([128, 3, C], F32)
        nc.gpsimd.partition_broadcast(bias_bc[:], brow_f[:])
        bv_bc = bias_bc[:, 0, :]
        bcp_bc = bias_bc[:, 1, :]
        bpj_bc = bias_bc[:, 2, :]

        # ------- activation tensors spanning stages 3-4 -------
        acts = ctx.enter_context(tc.tile_pool(name="acts", bufs=1))
        yT_sb = acts.tile([128, 6, 512], F32R)         # y cols x own q

        s123 = ctx.enter_context(ExitStack())
        acts13 = s123.enter_context(tc.tile_pool(name="acts13", bufs=1))
        hT_sb = acts13.tile([128, 6, T_kv], F32R)      # ln1(x) transposed
        qT_sb = acts13.tile([128, 6, 512], F32R)       # head-pair rows x own q

        # =================== stage 1: LN1 + transpose =======================
        with ExitStack() as s1:
            ln_pool = s1.enter_context(tc.tile_pool(name="ln", bufs=2))
            tp_ps = s1.enter_context(tc.tile_pool(name="tp_ps", bufs=3, space="PSUM"))
            for b2 in range(NB):
                x2_t = ln_pool.tile([128, 2, C], F32, tag="x")
                nc.sync.dma_start(
                    out=x2_t[:],
                    in_=x_in[b2 * 256:(b2 + 1) * 256, :].rearrange("(t p) c -> p t c", p=128))
                for tt in range(2):
                    ti = b2 * 2 + tt
                    x_t = x2_t[:, tt, :]
                    xg = x_t.rearrange("p (g d) -> p g d", g=3)
                    stats = ln_pool.tile([128, 3, 6], F32, tag="st")
                    for g in range(3):
                        nc.vector.bn_stats(out=stats[:, g, :], in_=xg[:, g, :])
                    mv = ln_pool.tile([128, 2], F32, tag="mv")
                    nc.vector.bn_aggr(out=mv[:], in_=stats[:])
                    rstd = ln_pool.tile([128, 1], F32, tag="rstd")
                    nc.scalar.activation(
                        out=rstd[:], in_=mv[:, 1:2],
                        func=mybir.ActivationFunctionType.Sqrt,
                        bias=eps_t[:], scale=1.0,
                    )
                    nc.vector.reciprocal(out=rstd[:], in_=rstd[:])
                    h_t = ln_pool.tile([128, C], F32R, tag="h")
                    nc.vector.tensor_scalar(
                        out=h_t[:], in0=x_t,
                        scalar1=mv[:, 0:1], scalar2=rstd[:],
                        op0=mybir.AluOpType.subtract, op1=mybir.AluOpType.mult,
                    )
                    for c in range(6):
                        pt = tp_ps.tile([128, 128], F32R, tag="tp")
                        nc.tensor.transpose(pt[:], h_t[:, c * 128:(c + 1) * 128], id_r[:])
                        evict(hT_sb[:, c, ti * 128:(ti + 1) * 128], pt[:])

        # ========== stages 2+3 per head-half: K/V/Q GEMMs + attention =======
        for ph in range(2):                 # heads 6*ph .. 6*ph+5
            with ExitStack() as s23:
                wkv_pool = s23.enter_context(tc.tile_pool(name="wkv", bufs=1))
                wq_pool = s23.enter_context(tc.tile_pool(name="wqs", bufs=2))
                kv_pool = s23.enter_context(tc.tile_pool(name="kv", bufs=1))
                att_pool = s23.enter_context(tc.tile_pool(name="att", bufs=5))
                nrm_pool = s23.enter_context(tc.tile_pool(name="nrm", bufs=2))
                gemm_ps = ExitStack()
                mm_ps = gemm_ps.enter_context(tc.tile_pool(name="mm_ps", bufs=3, space="PSUM"))

                co = ph * 384               # column offset of this head-half
                wk_t = wkv_pool.tile([128, 6, 384], F32R, tag="wk")
                wv_t = wkv_pool.tile([128, 6, 384], F32R, tag="wv")
                with tc.tile_pool(name="wkvs", bufs=2) as wkvs_pool:
                    for src_in, dst in ((wk_in, wk_t), (wv_in, wv_t)):
                        wstg = wkvs_pool.tile([128, 6, 384], F32, tag="wkvs")
                        nc.sync.dma_start(
                            out=wstg[:],
                            in_=src_in[:, co:co + 384].rearrange("(c k) n -> k c n", k=128))
                        for c in range(6):
                            cast(dst[:, c, :], wstg[:, c, :])

                kT_sb = kv_pool.tile([128, 3, T_kv], F32R, tag="kT")
                v_sb = kv_pool.tile([128, NTK, 6, 65], F32R, tag="v")
                for ti in range(NTK):
                    nc.vector.tensor_copy(v_sb[:, ti, :, 64], ones_col[:])

                # K GEMM (N=512)
                nblk512 = [(i * 512, 512) for i in range(T_kv // 512)]
                if T_kv % 512:
                    nblk512.append((T_kv - T_kv % 512, T_kv % 512))
                for tb, bw in nblk512:
                    for j in range(3):
                        pk = mm_ps.tile([128, 512], F32, tag="pk")
                        for c in range(6):
                            nc.tensor.matmul(
                                pk[:, 0:bw], wk_t[:, c, j * 128:(j + 1) * 128],
                                hT_sb[:, c, tb:tb + bw],
                                start=(c == 0), stop=(c == 5),
                            )
                        nc.vector.tensor_scalar(
                            out=kT_sb[:, j, tb:tb + bw], in0=pk[:, 0:bw],
                            scalar1=bk_sb[:, 3 * ph + j:3 * ph + j + 1], scalar2=None,
                            op0=mybir.AluOpType.add,
                        )
                # V GEMM (natural layout)
                for ti in range(NTK):
                    pv = mm_ps.tile([128, 384], F32, tag="pv")
                    for c in range(6):
                        nc.tensor.matmul(
                            pv[:], hT_sb[:, c, ti * 128:(ti + 1) * 128],
                            wv_t[:, c, :],
                            start=(c == 0), stop=(c == 5),
                        )
                    nc.vector.tensor_tensor(
                        out=v_sb[:, ti, :, 0:64],
                        in0=pv[:].rearrange("p (h d) -> p h d", d=64),
                        in1=bv_bc[:, co:co + 384].rearrange("p (h d) -> p h d", d=64),
                        op=mybir.AluOpType.add,
                    )
                # Q GEMM for own strips (both strips in one N=512 matmul)
                tbA, tbB = blkA * 256, blkB * 256
                for j in range(3):
                    jj = 3 * ph + j
                    wq_s = wq_pool.tile([128, 6, 128], F32, tag="wqs")
                    nc.sync.dma_start(
                        out=wq_s[:],
                        in_=wq_in[:, jj * 128:(jj + 1) * 128].rearrange(
                            "(c k) n -> k c n", k=128))
                    wq_t = wq_pool.tile([128, 6, 128], F32R, tag="wqr")
                    for c in range(6):
                        cast(wq_t[:, c, :], wq_s[:, c, :])
                    pq = mm_ps.tile([128, 512], F32, tag="pk")
                    for c in range(6):
                        rhs = bass.AP(
                            tensor=hT_sb[:, c, :].tensor,
                            offset=hT_sb[:, c, tbA:tbA + 1].offset,
                            ap=[list(p) for p in hT_sb[:, c, :].ap[:1]]
                            + [[hT_sb[:, c, :].ap[-1][0] * (tbB - tbA), 2],
                               [hT_sb[:, c, :].ap[-1][0], 256]],
                        )
                        nc.tensor.matmul(
                            pq[:], wq_t[:, c, :], rhs,
                            start=(c == 0), stop=(c == 5),
                        )
                    nc.vector.tensor_scalar(
                        out=qT_sb[:, jj, :], in0=pq[:],
                        scalar1=bq_sb[:, jj:jj + 1], scalar2=None,
                        op0=mybir.AluOpType.add,
                    )

                gemm_ps.close()
                att_scope = ExitStack()
                att_ps = att_scope.enter_context(tc.tile_pool(name="att_ps", bufs=5, space="PSUM"))
                yt_ps = att_scope.enter_context(tc.tile_pool(name="yt_ps", bufs=2, space="PSUM"))
                # ---- attention for heads of this half ----
                # chunks < n_sh apply to both strips (N=512); rest strip-B only
                n_sh = 2 * (sA + 1)
                n_all = 2 * (sB + 1)
                for hh in range(6):
                    h = 6 * ph + hh
                    j, po = hh // 2, 64 * (hh % 2)
                    jj = h // 2
                    kT_h = kT_sb[po:po + 64, j, :]
                    qT_h = qT_sb[64 * (h % 2):64 * (h % 2) + 64, jj, :]
                    yt = yt_ps.tile([65, 512], F32, tag="yt")
                    for kc in range(n_all):
                        shared = kc < n_sh
                        ww = 512 if shared else 256
                        qs = 0 if shared else 256
                        pa = att_ps.tile([128, 512], F32, tag="pa")
                        nc.tensor.matmul(
                            pa[:, 0:ww], kT_h[:, kc * 128:(kc + 1) * 128],
                            qT_h[:, qs:qs + ww],
                            start=True, stop=True,
                        )
                        at = att_pool.tile([128, 512], F32R, tag="at")
                        nc.scalar.activation(
                            out=at[:, 0:ww], in_=pa[:, 0:ww],
                            func=mybir.ActivationFunctionType.Exp,
                        )
                        if kc in (2 * sA, 2 * sA + 1):
                            nc.vector.tensor_mul(
                                at[:, 0:256], at[:, 0:256], mask_t[:, kc - 2 * sA, :])
                        if kc in (2 * sB, 2 * sB + 1):
                            # strip-B columns live at 256:512 when shared, 0:256 when B-only
                            boff = 256 if shared else 0
                            nc.vector.tensor_mul(
                                at[:, boff:boff + 256],
                                at[:, boff:boff + 256],
                                mask_t[:, kc - 2 * sB, :])
                        nc.tensor.matmul(
                            yt[0:65, qs:qs + ww], v_sb[:, kc, hh, 0:65], at[:, 0:ww],
                            start=(kc == 0), stop=(kc == n_all - 1),
                            skip_group_check=True,
                        )
                    sume = nrm_pool.tile([1, 512], F32, tag="sume")
                    nc.vector.tensor_copy(sume[:], yt[64:65, :])
                    bcast = nrm_pool.tile([64, 512], F32, tag="bcast")
                    nc.gpsimd.partition_broadcast(bcast[:], sume[:])
                    nc.vector.reciprocal(out=bcast[:], in_=bcast[:])
                    nc.vector.tensor_mul(
                        yT_sb[po:po + 64, jj, :], yt[0:64, :], bcast[:],
                    )
                att_scope.close()

        s123.close()  # free hT/qT SBUF before the MLP stages

        # =================== stages 4-6: c_proj, MLP ========================
        with ExitStack() as s46:
            wcp_pool = s46.enter_context(tc.tile_pool(name="wcp", bufs=1))
            act46 = s46.enter_context(tc.tile_pool(name="act46", bufs=1))
            ln2_pool = s46.enter_context(tc.tile_pool(name="ln2", bufs=2))
            stream_pool = s46.enter_context(tc.tile_pool(name="stream", bufs=2))
            out_pool = s46.enter_context(tc.tile_pool(name="outp", bufs=3))

            # c_proj weights resident fp32r
            wcp_t = wcp_pool.tile([128, 6, C], F32R)
            with tc.tile_pool(name="wcps", bufs=1) as wcps_pool:
                wstg = wcps_pool.tile([128, 6, C], F32, tag="wcps")
                nc.sync.dma_start(out=wstg[:], in_=wcp_in[:].rearrange("(j k) n -> k j n", k=128))
                for j in range(6):
                    cast(wcp_t[:, j, :], wstg[:, j, :])

            x1_sb = act46.tile([128, 4, C], F32)
            h2T_sb = act46.tile([128, 6, 512], F32R)
            gT_sb = act46.tile([128, 24, 512], F32R)

            own_rows = (sA * 256, sA * 256 + 128, sB * 256, sB * 256 + 128)
            # ---- stage 4: c_proj + residual + LN2 + transpose ----
            s4 = ExitStack()
            tp2_ps = s4.enter_context(tc.tile_pool(name="tp2_ps", bufs=2, space="PSUM"))
            cp_ps = s4.enter_context(tc.tile_pool(name="cp_ps", bufs=2, space="PSUM"))
            for m in range(4):
                pp = []
                for i in range(2):
                    pp_i = cp_ps.tile([128, 384], F32, tag=f"cp{i}")
                    pp.append(pp_i)
                for half in range(2):
                    for j in range(6):
                        nc.tensor.matmul(
                            pp[half][:],
                            yT_sb[:, j, m * 128:(m + 1) * 128],
                            wcp_t[:, j, half * 384:(half + 1) * 384],
                            start=(j == 0), stop=(j == 5),
                        )
                x_own = ln2_pool.tile([128, C], F32, tag="xo")
                nc.sync.dma_start(out=x_own[:], in_=x_in[own_rows[m]:own_rows[m] + 128, :])
                nc.vector.tensor_add(x_own[:], x_own[:], bcp_bc[:])
                for half in range(2):
                    nc.vector.tensor_add(
                        x1_sb[:, m, half * 384:(half + 1) * 384],
                        pp[half][:], x_own[:, half * 384:(half + 1) * 384],
                    )
                # LN2
                x1g = x1_sb[:, m, :].rearrange("p (g d) -> p g d", g=3)
                stats = ln2_pool.tile([128, 3, 6], F32, tag="st2")
                for g in range(3):
                    nc.vector.bn_stats(out=stats[:, g, :], in_=x1g[:, g, :])
                mv = ln2_pool.tile([128, 2], F32, tag="mv2")
                nc.vector.bn_aggr(out=mv[:], in_=stats[:])
                rstd = ln2_pool.tile([128, 1], F32, tag="rstd2")
                nc.scalar.activation(
                    out=rstd[:], in_=mv[:, 1:2],
                    func=mybir.ActivationFunctionType.Sqrt,
                    bias=eps_t[:], scale=1.0,
                )
                nc.vector.reciprocal(out=rstd[:], in_=rstd[:])
                h2 = ln2_pool.tile([128, C], F32R, tag="h2")
                nc.vector.tensor_scalar(
                    out=h2[:], in0=x1_sb[:, m, :],
                    scalar1=mv[:, 0:1], scalar2=rstd[:],
                    op0=mybir.AluOpType.subtract, op1=mybir.AluOpType.mult,
                )
                for c in range(6):
                    pt = tp2_ps.tile([128, 128], F32R, tag="tp2")
                    nc.tensor.transpose(pt[:], h2[:, c * 128:(c + 1) * 128], id_r[:])
                    evict(h2T_sb[:, c, m * 128:(m + 1) * 128], pt[:])

            s4.close()
            # ---- stage 5: fc + gelu ----
            s5 = ExitStack()
            pf_ps = s5.enter_context(tc.tile_pool(name="pf_ps", bufs=3, space="PSUM"))
            for f in range(24):
                wfc_s = stream_pool.tile([128, 6, 128], F32, tag="wfc_s")
                nc.sync.dma_start(
                    out=wfc_s[:],
                    in_=wfc_in[:, f * 128:(f + 1) * 128].rearrange("(c k) n -> k c n", k=128),
                )
                wfc_t = stream_pool.tile([128, 6, 128], F32R, tag="wfc_r")
                for c in range(6):
                    cast(wfc_t[:, c, :], wfc_s[:, c, :])
                pf = pf_ps.tile([128, 512], F32, tag="pf")
                for c in range(6):
                    nc.tensor.matmul(
                        pf[:], wfc_t[:, c, :], h2T_sb[:, c, :],
                        start=(c == 0), stop=(c == 5),
                    )
                nc.scalar.activation(
                    out=gT_sb[:, f, :], in_=pf[:],
                    func=mybir.ActivationFunctionType.Gelu_apprx_tanh,
                    bias=bfc_sb[:, f:f + 1], scale=1.0,
                )

            s5.close()
            # ---- stage 6: proj + residual + store (all 4 blocks, one wpj pass) ----
            s6 = ExitStack()
            pj_ps = s6.enter_context(tc.tile_pool(name="pj_ps", bufs=1, space="PSUM"))
            pj = []
            for i in range(8):
                pj_i = pj_ps.tile([128, 384], F32, tag=f"pj{i}")
                pj.append(pj_i)
            for f in range(24):
                wpj_s = stream_pool.tile([128, C], F32, tag="wpj_s")
                nc.sync.dma_start(out=wpj_s[:], in_=wpj_in[f * 128:(f + 1) * 128, :])
                wpj_t = stream_pool.tile([128, C], F32R, tag="wpj_r")
                cast(wpj_t[:, 0:384], wpj_s[:, 0:384])
                cast(wpj_t[:, 384:768], wpj_s[:, 384:768])
                for m in range(4):
                    for half in range(2):
                        nc.tensor.matmul(
                            pj[m * 2 + half][:],
                            gT_sb[:, f, m * 128:(m + 1) * 128],
                            wpj_t[:, half * 384:(half + 1) * 384],
                            start=(f == 0), stop=(f == 23),
                        )
            for m in range(4):
                o_t = out_pool.tile([128, C], F32, tag="o")
                for half in range(2):
                    nc.vector.tensor_add(
                        o_t[:, half * 384:(half + 1) * 384],
                        pj[m * 2 + half][:],
                        x1_sb[:, m, half * 384:(half + 1) * 384],
                    )
                nc.vector.tensor_add(o_t[:], o_t[:], bpj_bc[:])
                nc.sync.dma_start(out=out_dram[m * 128:(m + 1) * 128, :], in_=o_t[:])
            s6.close()


# ---------------------------------------------------------------------------
# Runner
# ---------------------------------------------------------------------------
def _make_runner(nc):
    partition_name = nc.partition_id_tensor.name if nc.partition_id_tensor else None
    in_names, out_names, out_avals, zero_outs = [], [], [], []
    for alloc in nc.m.functions[0].allocations:
        if not isinstance(alloc, mybir.MemoryLocationSet):
            continue
        name = alloc.memorylocations[0].name
        if alloc.kind == "ExternalInput":
            if name != partition_name:
                in_names.append(name)
        elif alloc.kind == "ExternalOutput":
            out_names.append(name)
            shape = tuple(alloc.tensor_shape)
            dtype = mybir.dt.np(alloc.dtype)
            out_avals.append(jax.core.ShapedArray(shape, dtype))
            zero_outs.append(np.zeros(shape, dtype))
    n_params = len(in_names)
    all_names = list(in_names) + list(out_names)
    if partition_name is not None:
        all_names.append(partition_name)

    def _body(*args):
        operands = list(args)
        if partition_name is not None:
            operands.append(partition_id_tensor())
        outs = _bass_exec_p.bind(
            *operands,
            out_avals=tuple(out_avals),
            in_names=tuple(all_names),
            out_names=tuple(out_names),
            lowering_input_output_aliases=(),
            sim_require_finite=True,
            sim_require_nnan=True,
            nc=nc,
        )
        return tuple(outs)

    donate = tuple(range(n_params, n_params + len(out_names)))
    jitted = jax.jit(_body, donate_argnums=donate, keep_unused=True)
    return jitted, in_names, out_names, zero_outs


@functools.lru_cache(maxsize=None)
def _get_runners():
    install_neuronx_cc_hook()
    runners = []
    for r in range(4):
        nc = build_rank_program(r)
        runners.append(_make_runner(nc))
    return runners


def _prep_core_inputs(x, ln1_w, ln1_b, c_attn_w, c_attn_b, c_proj_w, c_proj_b,
                      ln2_w, ln2_b, fc_w, fc_b, proj_w, proj_b):
    """Fold LN affines into weights; split qkv. Returns shared weight dict."""
    f32 = np.float32
    wqkv = (ln1_w[:, None] * c_attn_w).astype(f32)
    bqkv = (c_attn_b + ln1_b @ c_attn_w).astype(f32)
    scale = f32(1.0 / np.sqrt(HD))
    shared = {
        "wq": np.ascontiguousarray(wqkv[:, 0:C] * scale),
        "wk": np.ascontiguousarray(wqkv[:, C:2 * C]),
        "wv": np.ascontiguousarray(wqkv[:, 2 * C:3 * C]),
        "bq": np.ascontiguousarray(bqkv[0:C] * scale),
        "bk": np.ascontiguousarray(bqkv[C:2 * C]),
        "bv": np.ascontiguousarray(bqkv[2 * C:3 * C]),
        "wcp": np.ascontiguousarray(c_proj_w.astype(f32)),
        "bcp": np.ascontiguousarray(c_proj_b.astype(f32)),
        "wfc": np.ascontiguousarray((ln2_w[:, None] * fc_w).astype(f32)),
        "bfc": np.ascontiguousarray((fc_b + ln2_b @ fc_w).astype(f32)),
        "wpj": np.ascontiguousarray(proj_w.astype(f32)),
        "bpj": np.ascontiguousarray(proj_b.astype(f32)),
    }
    return shared


def _dispatch_all(inputs):
    """Dispatch the 8 per-core executions asynchronously; return futures."""
    runners = _get_runners()
    devices = jax.devices()
    shared = _prep_core_inputs(**{k: np.asarray(v) for k, v in inputs.items()})
    x = np.asarray(inputs["x"], dtype=np.float32)
    futs = []
    for c in range(8):
        b, r = c // 4, c % 4
        jitted, in_names, out_names, zero_outs = runners[r]
        dev = devices[c]
        per_core = dict(shared)
        per_core["x"] = np.ascontiguousarray(x[b])
        args = [jax.device_put(per_core[n], dev) for n in in_names]
        args += [jax.device_put(z, dev) for z in zero_outs]
        futs.append((c, out_names, jitted(*args)))
    return futs


def kernel(**inputs) -> np.ndarray:
    futs = _dispatch_all(inputs)
    out = np.empty((B, T, C), dtype=np.float32)
    for c, out_names, fut in futs:
        b, r = c // 4, c % 4
        res = np.asarray(fut[out_names.index("out")])
        out[b, 256 * r:256 * r + 256] = res[0:256]
        out[b, 256 * (7 - r):256 * (7 - r) + 256] = res[256:512]
    return out
